# revision 2
# baseline (speedup 1.0000x reference)
"""GQA attention kernel for Trainium2, tensor-parallel over heads on 8 cores.

Problem: B=1, T=2048, EMB=4096, H=32 query heads, G=8 KV groups, D=128.
Reference: q/k/v projections -> per-head RMS norm (q,k) -> RoPE (q,k) ->
causal GQA attention -> out projection.

Sharding: core c owns query heads [4c, 4c+4) and KV group c.  Each core
computes a partial output for its heads; host sums the 8 partials (the
all-reduce of the module's TP scheme, done on host since full I/O is
required anyway).

v2 changes vs v1 (709us baseline):
  - Phase B software-pipelined: S matmuls run 2 blocks ahead of the
    exp->den/ctx chain so the PE never stalls on ACT latency, which also
    keeps the PE at its max p-state clock.
  - Softmax denominator broadcast via an all-ones [128,128] stationary
    matmul (accumulating over key blocks), replacing the [1,512]
    denominator row + reciprocal + K=1 broadcast matmul + 2 ACT copies.
  - All weights pre-swizzled host-side into partition-major layouts so
    each weight tensor is ONE contiguous DMA post (the baseline's 70+
    serial posts on the Sync queue cost ~45us of startup stall).
  - cos/sin tables merged into one [T,512] table (1 post per strip).
  - Phase C writes bf16 partials via one [128,4096] DMA per row block.
"""

import numpy as np
import ml_dtypes
from contextlib import ExitStack

import concourse.bass as bass
import concourse.bacc as bacc
import concourse.mybir as mybir
from concourse.tile import TileContext
from concourse.bass_utils import run_bass_kernel_spmd
from concourse.masks import make_identity

EMB, H, G, D, T = 4096, 32, 8, 128, 2048
EPS = 1e-6
NCORES = 8
HP = H // NCORES          # 4 query heads per core
NT = T // 128             # 16 t-tiles
NE = EMB // 128           # 32 e-tiles
NO = EMB // 512           # 8 output column tiles
QW = HP * D               # 512 = q width per core
KVW = 2 * D               # 256 = k|v width per core
SM_SCALE = 1.0 / float(np.sqrt(D))
NEG = -1e9

F32 = mybir.dt.float32
BF16 = mybir.dt.bfloat16
BF = ml_dtypes.bfloat16

_prog_cache = {}


def _build_program():
    nc = bacc.Bacc()

    xT_d = nc.declare_dram_parameter("xT", [NT * 128, NE * 128], BF16, isOutput=False)
    wq_d = nc.declare_dram_parameter("wq", [128, NE * QW], BF16, isOutput=False)
    wkv_d = nc.declare_dram_parameter("wkv", [128, NE * KVW], BF16, isOutput=False)
    wo_d = nc.declare_dram_parameter("wo", [128, HP * EMB], BF16, isOutput=False)
    cs_d = nc.declare_dram_parameter("cs", [NT * 128, 512], F32, isOutput=False)
    mask_d = nc.declare_dram_parameter("maskT", [128, 896], F32, isOutput=False)
    bias_d = nc.declare_dram_parameter("biasb", [128, QW + KVW], F32, isOutput=False)
    out_d = nc.declare_dram_parameter("out", [T, EMB], BF16, isOutput=True)

    with TileContext(nc) as tc, ExitStack() as ctx:
        consts = ctx.enter_context(tc.tile_pool(name="consts", bufs=1))
        wpool = ctx.enter_context(tc.tile_pool(name="wpool", bufs=1))
        xpool = ctx.enter_context(tc.tile_pool(name="xpool", bufs=2))
        cspool = ctx.enter_context(tc.tile_pool(name="cspool", bufs=2))
        scratch = ctx.enter_context(tc.tile_pool(name="scratch", bufs=3))
        small = ctx.enter_context(tc.tile_pool(name="small", bufs=4))
        ppool = ctx.enter_context(tc.tile_pool(name="ppool", bufs=3))
        epool = ctx.enter_context(tc.tile_pool(name="epool", bufs=2))
        opool = ctx.enter_context(tc.tile_pool(name="opool", bufs=2))
        resid = ctx.enter_context(tc.tile_pool(name="resid", bufs=1))
        psA = ctx.enter_context(tc.tile_pool(name="psA", bufs=3, space="PSUM"))
        psB = ctx.enter_context(tc.tile_pool(name="psB", bufs=2, space="PSUM"))
        psC = ctx.enter_context(tc.tile_pool(name="psC", bufs=2, space="PSUM"))

        # strip 0 inputs first so phase A can start while weights stream in
        xstrip0 = xpool.tile([128, NE * 128], BF16, tag="xstrip", name="xstrip0")
        nc.sync.dma_start(out=xstrip0[:, 0:2048], in_=xT_d[0:128, 0:2048])
        nc.sync.dma_start(out=xstrip0[:, 2048:4096], in_=xT_d[0:128, 2048:4096])
        cs0 = cspool.tile([128, 512], F32, tag="cs", name="cs0")
        nc.sync.dma_start(out=cs0, in_=cs_d[0:128, :])

        # resident weights: one contiguous DMA each (pre-swizzled on host)
        wq_sb = wpool.tile([128, NE * QW], BF16, tag="wq", name="wq")
        nc.sync.dma_start(out=wq_sb[:, 0:NE * QW // 2], in_=wq_d[:, 0:NE * QW // 2])
        nc.sync.dma_start(out=wq_sb[:, NE * QW // 2:], in_=wq_d[:, NE * QW // 2:])
        wkv_sb = wpool.tile([128, NE * KVW], BF16, tag="wkv", name="wkv")
        nc.sync.dma_start(out=wkv_sb, in_=wkv_d[:, :])

        # constants
        ident = consts.tile([128, 128], BF16, tag="ident", name="ident")
        make_identity(nc, ident)
        ones_sq = consts.tile([128, 128], BF16, tag="ones_sq", name="ones_sq")
        nc.vector.memset(ones_sq, 1.0)
        eps_t = consts.tile([128, 1], F32, tag="eps", name="eps")
        nc.vector.memset(eps_t, EPS)
        mask_sb = consts.tile([128, 896], F32, tag="mask", name="mask")
        nc.sync.dma_start(out=mask_sb, in_=mask_d[:, :])
        bias_sb = consts.tile([128, QW + KVW], F32, tag="bias", name="bias")
        nc.sync.dma_start(out=bias_sb, in_=bias_d[:, :])

        # out-proj weights last (not needed until phase C)
        wo_sb = wpool.tile([128, HP * EMB], BF16, tag="wo", name="wo")
        nc.sync.dma_start(out=wo_sb[:, 0:HP * EMB // 2], in_=wo_d[:, 0:HP * EMB // 2])
        nc.sync.dma_start(out=wo_sb[:, HP * EMB // 2:], in_=wo_d[:, HP * EMB // 2:])

        # resident activations
        qT = [resid.tile([128, T], BF16, tag=f"qT{h}", name=f"qT{h}") for h in range(HP)]
        kT = resid.tile([128, T], BF16, tag="kT", name="kT")
        vsb = [resid.tile([128, 128], BF16, tag=f"v{j}", name=f"v{j}") for j in range(NT)]
        ctxT = [resid.tile([128, T], BF16, tag=f"ctxT{h}", name=f"ctxT{h}") for h in range(HP)]

        # ---------------- Phase A: projections + rms + rope + transpose ----
        for it in range(NT):
            if it == 0:
                xstrip, cs = xstrip0, cs0
            else:
                xstrip = xpool.tile([128, NE * 128], BF16, tag="xstrip",
                                    name=f"xstrip{it}")
                r0, r1 = it * 128, (it + 1) * 128
                nc.sync.dma_start(out=xstrip[:, 0:2048], in_=xT_d[r0:r1, 0:2048])
                nc.sync.dma_start(out=xstrip[:, 2048:4096], in_=xT_d[r0:r1, 2048:4096])
                cs = cspool.tile([128, 512], F32, tag="cs", name=f"cs{it}")
                nc.sync.dma_start(out=cs, in_=cs_d[r0:r1, :])

            q_ps = psA.tile([128, QW], F32, tag="m", name="q_ps")
            kv_ps = psB.tile([128, KVW], F32, tag="c", name="kv_ps")
            for e in range(NE):
                xt = xstrip[:, e * 128:(e + 1) * 128]
                nc.tensor.matmul(q_ps, xt, wq_sb[:, e * QW:(e + 1) * QW],
                                 start=(e == 0), stop=(e == NE - 1))
                nc.tensor.matmul(kv_ps, xt, wkv_sb[:, e * KVW:(e + 1) * KVW],
                                 start=(e == 0), stop=(e == NE - 1))
            nc.vector.tensor_add(q_ps, q_ps, bias_sb[:, 0:QW])
            nc.vector.tensor_add(kv_ps, kv_ps, bias_sb[:, QW:QW + KVW])

            for b in range(HP + 1):  # 0..3 q heads, 4 = k
                if b < HP:
                    src = q_ps[:, b * 128:(b + 1) * 128]
                    c_t, s_t = cs[:, 0:128], cs[:, 128:256]
                else:
                    src = kv_ps[:, 0:128]
                    c_t, s_t = cs[:, 256:384], cs[:, 384:512]
                sqout = scratch.tile([128, 128], F32, tag="sqout", name="sqout")
                sqacc = small.tile([128, 1], F32, tag="sqacc", name="sqacc")
                nc.scalar.activation(
                    out=sqout, in_=src,
                    func=mybir.ActivationFunctionType.Square,
                    accum_out=sqacc,
                )
                rstd = small.tile([128, 1], F32, tag="rstd", name="rstd")
                nc.scalar.activation(
                    out=rstd, in_=sqacc,
                    func=mybir.ActivationFunctionType.Sqrt,
                    bias=eps_t, scale=1.0 / D,
                )
                nc.vector.reciprocal(rstd, rstd)
                # rope: out1 = x1*c1 - x2*s1 ; out2 = x2*c2 + x1*s2
                rt = scratch.tile([128, 128], F32, tag="rt", name="rt")
                m1 = scratch.tile([128, 64], F32, tag="m1", name="m1")
                nc.vector.tensor_mul(rt[:, 0:64], src[:, 0:64], c_t[:, 0:64])
                nc.vector.tensor_mul(m1, src[:, 64:128], s_t[:, 0:64])
                nc.vector.tensor_sub(rt[:, 0:64], rt[:, 0:64], m1)
                m2 = scratch.tile([128, 64], F32, tag="m2", name="m2")
                nc.vector.tensor_mul(rt[:, 64:128], src[:, 64:128], c_t[:, 64:128])
                nc.vector.tensor_mul(m2, src[:, 0:64], s_t[:, 64:128])
                nc.vector.tensor_add(rt[:, 64:128], rt[:, 64:128], m2)
                rb = scratch.tile([128, 128], BF16, tag="rb", name="rb")
                nc.vector.tensor_scalar_mul(rb, rt, rstd)
                tp = psC.tile([128, 128], BF16, tag="d", name="tp")
                nc.tensor.transpose(tp, rb, ident)
                dst = qT[b] if b < HP else kT
                nc.scalar.copy(out=dst[:, it * 128:(it + 1) * 128], in_=tp)
            # v
            nc.scalar.copy(out=vsb[it], in_=kv_ps[:, 128:256])

        # ---------------- Phase B: attention -------------------------------
        for h in range(HP):
            for si in range(T // 512):
                njb = 4 * si + 4
                qslice = qT[h][:, si * 512:(si + 1) * 512]
                ctx_ps = psB.tile([128, 512], F32, tag="c", name="ctx_ps")
                den_ps = psC.tile([128, 512], F32, tag="d", name="den_ps")

                def emit_s(jb):
                    s_ps = psA.tile([128, 512], F32, tag="m", name="s_ps")
                    nc.tensor.matmul(
                        s_ps, kT[:, jb * 128:(jb + 1) * 128], qslice,
                        start=True, stop=True,
                    )
                    kk = jb - 4 * si
                    if kk >= 0:  # diagonal (partially masked) block
                        off = 384 - 128 * kk
                        nc.vector.tensor_add(s_ps, s_ps, mask_sb[:, off:off + 512])
                    return s_ps

                s_tiles = [emit_s(0)]
                if njb > 1:
                    s_tiles.append(emit_s(1))
                for jb in range(njb):
                    p_t = ppool.tile([128, 512], BF16, tag="pt", name="pt")
                    nc.scalar.activation(
                        out=p_t, in_=s_tiles[jb],
                        func=mybir.ActivationFunctionType.Exp,
                        scale=SM_SCALE,
                    )
                    if jb + 2 < njb:
                        s_tiles.append(emit_s(jb + 2))
                    nc.tensor.matmul(den_ps, ones_sq, p_t,
                                     start=(jb == 0), stop=(jb == njb - 1))
                    nc.tensor.matmul(ctx_ps, vsb[jb], p_t,
                                     start=(jb == 0), stop=(jb == njb - 1))
                rden = epool.tile([128, 512], F32, tag="rden", name="rden")
                nc.vector.reciprocal(rden, den_ps)
                nc.vector.tensor_mul(
                    ctxT[h][:, si * 512:(si + 1) * 512], ctx_ps, rden)

        # ---------------- Phase C: out projection --------------------------
        for it in range(NT):
            osb = opool.tile([128, EMB], BF16, tag="osb", name="osb")
            for ot in range(NO):
                o_ps = psA.tile([128, 512], F32, tag="m", name="o_ps")
                for hh in range(HP):
                    nc.tensor.matmul(
                        o_ps,
                        ctxT[hh][:, it * 128:(it + 1) * 128],
                        wo_sb[:, hh * EMB + ot * 512:hh * EMB + (ot + 1) * 512],
                        start=(hh == 0), stop=(hh == HP - 1),
                    )
                nc.scalar.copy(out=osb[:, ot * 512:(ot + 1) * 512], in_=o_ps)
            nc.sync.dma_start(
                out=out_d[it * 128:(it + 1) * 128, :], in_=osb)

    return nc


def _prep_inputs(x, mask, cos, sin, wq, bq, wk, bk, wv, bv, wo, q_scale, k_scale):
    x2 = np.asarray(x, dtype=np.float32).reshape(T, EMB)
    # strip layout: row (it*128 + p), col (eb*128 + t) holds x[it*128+t, eb*128+p]
    xTt = x2.reshape(NT, 128, NE, 128).transpose(0, 3, 2, 1)
    xTt = np.ascontiguousarray(xTt).reshape(NT * 128, NE * 128).astype(BF)

    qs = np.asarray(q_scale, dtype=np.float32)
    ks = np.asarray(k_scale, dtype=np.float32)
    qs_rot = np.concatenate([qs[64:], qs[:64]])
    ks_rot = np.concatenate([ks[64:], ks[:64]])
    cos = np.asarray(cos, dtype=np.float32)
    sin = np.asarray(sin, dtype=np.float32)
    cs = np.concatenate([cos * qs[None, :], sin * qs_rot[None, :],
                         cos * ks[None, :], sin * ks_rot[None, :]], axis=1)
    cs = np.ascontiguousarray(cs, dtype=np.float32)

    jj = np.arange(128)[:, None]
    cc = np.arange(896)[None, :]
    maskT = np.where(jj > cc - 384, NEG, 0.0).astype(np.float32)

    wq = np.asarray(wq, dtype=np.float32)
    wk = np.asarray(wk, dtype=np.float32)
    wv = np.asarray(wv, dtype=np.float32)
    wo = np.asarray(wo, dtype=np.float32)
    bq = np.asarray(bq, dtype=np.float32)
    bk = np.asarray(bk, dtype=np.float32)
    bv = np.asarray(bv, dtype=np.float32)

    in_maps = []
    for c in range(NCORES):
        # [p, e*QW + o] = wq[e*128 + p, c*QW + o]
        wq_c = wq[:, c * QW:(c + 1) * QW].reshape(NE, 128, QW)
        wq_c = np.ascontiguousarray(wq_c.transpose(1, 0, 2)).reshape(128, NE * QW)
        wkv_c = np.concatenate(
            [wk[:, c * D:(c + 1) * D], wv[:, c * D:(c + 1) * D]], axis=1)
        wkv_c = wkv_c.reshape(NE, 128, KVW)
        wkv_c = np.ascontiguousarray(wkv_c.transpose(1, 0, 2)).reshape(128, NE * KVW)
        # [p, h*EMB + col] = wo[c*QW + h*128 + p, col]
        wo_c = wo[c * QW:(c + 1) * QW, :].reshape(HP, 128, EMB)
        wo_c = np.ascontiguousarray(wo_c.transpose(1, 0, 2)).reshape(128, HP * EMB)
        bias_c = np.broadcast_to(
            np.concatenate([bq[c * QW:(c + 1) * QW],
                            bk[c * D:(c + 1) * D], bv[c * D:(c + 1) * D]]),
            (128, QW + KVW))
        in_maps.append({
            "xT": xTt,
            "wq": wq_c.astype(BF),
            "wkv": wkv_c.astype(BF),
            "wo": wo_c.astype(BF),
            "cs": cs,
            "maskT": maskT,
            "biasb": np.ascontiguousarray(bias_c, dtype=np.float32),
        })
    return in_maps


def _get_program():
    if "nc" not in _prog_cache:
        nc = _build_program()
        if not nc.is_finalized():
            nc.finalize()
        _prog_cache["nc"] = nc
    return _prog_cache["nc"]


def kernel(**inputs):
    in_maps = _prep_inputs(**inputs)
    nc = _get_program()
    res = run_bass_kernel_spmd(nc, in_maps, list(range(NCORES)))
    out = np.zeros((T, EMB), dtype=np.float32)
    for r in res.results:
        out += np.asarray(r["out"], dtype=np.float32)
    return out.reshape(1, T, EMB)


# revision 3
# speedup vs baseline: 1.0101x; 1.0101x over previous
"""GQA attention kernel for Trainium2, tensor-parallel over heads on 8 cores.

Problem: B=1, T=2048, EMB=4096, H=32 query heads, G=8 KV groups, D=128.
Reference: q/k/v projections -> per-head RMS norm (q,k) -> RoPE (q,k) ->
causal GQA attention -> out projection.

Sharding: core c owns query heads [4c, 4c+4) and KV group c.  Each core
computes a partial output for its heads; host sums the 8 partials (the
all-reduce of the module's TP scheme, done on host since full I/O is
required anyway).

v3 (487us v2 -> target ~420us):
  - Phase B flattened: one software pipeline per 512-query slice across
    all 4 heads (S matmuls 2 blocks ahead), si-outer loop; the group
    epilogue (reciprocal+normalize) lands on the DVE queue after the
    next group's mask add, off the critical path.
  - Phase C interleaved per si-group right after its 4 head-groups:
    output DMA drains during attention instead of all at the end.
  - reciprocal_approx_fast (5x faster DVE reciprocal, 18-bit) for both
    softmax denominators and rms rstd.
  - Phase C PSUM->SBUF copies alternate between ACT and DVE.
  - Weight DMAs split into ~512KB posts interleaved in e-order so the
    first projection matmuls start as soon as their slice arrives.
"""

import numpy as np
import ml_dtypes
from contextlib import ExitStack

import concourse.bass as bass
import concourse.bacc as bacc
import concourse.mybir as mybir
from concourse.tile import TileContext
from concourse.bass_utils import run_bass_kernel_spmd
from concourse.masks import make_identity

EMB, H, G, D, T = 4096, 32, 8, 128, 2048
EPS = 1e-6
NCORES = 8
HP = H // NCORES          # 4 query heads per core
NT = T // 128             # 16 t-tiles
NE = EMB // 128           # 32 e-tiles
NO = EMB // 512           # 8 output column tiles
QW = HP * D               # 512 = q width per core
KVW = 2 * D               # 256 = k|v width per core
SM_SCALE = 1.0 / float(np.sqrt(D))
NEG = -1e9

F32 = mybir.dt.float32
BF16 = mybir.dt.bfloat16
BF = ml_dtypes.bfloat16

_prog_cache = {}


def _build_program():
    nc = bacc.Bacc()

    xT_d = nc.declare_dram_parameter("xT", [NT * 128, NE * 128], BF16, isOutput=False)
    wq_d = nc.declare_dram_parameter("wq", [128, NE * QW], BF16, isOutput=False)
    wkv_d = nc.declare_dram_parameter("wkv", [128, NE * KVW], BF16, isOutput=False)
    wo_d = nc.declare_dram_parameter("wo", [128, HP * EMB], BF16, isOutput=False)
    cs_d = nc.declare_dram_parameter("cs", [NT * 128, 512], F32, isOutput=False)
    mask_d = nc.declare_dram_parameter("maskT", [128, 896], F32, isOutput=False)
    bias_d = nc.declare_dram_parameter("biasb", [128, QW + KVW], F32, isOutput=False)
    out_d = nc.declare_dram_parameter("out", [T, EMB], BF16, isOutput=True)

    with TileContext(nc) as tc, ExitStack() as ctx:
        consts = ctx.enter_context(tc.tile_pool(name="consts", bufs=1))
        wpool = ctx.enter_context(tc.tile_pool(name="wpool", bufs=1))
        xpool = ctx.enter_context(tc.tile_pool(name="xpool", bufs=2))
        cspool = ctx.enter_context(tc.tile_pool(name="cspool", bufs=2))
        scratch = ctx.enter_context(tc.tile_pool(name="scratch", bufs=3))
        small = ctx.enter_context(tc.tile_pool(name="small", bufs=4))
        ppool = ctx.enter_context(tc.tile_pool(name="ppool", bufs=3))
        epool = ctx.enter_context(tc.tile_pool(name="epool", bufs=2))
        opool = ctx.enter_context(tc.tile_pool(name="opool", bufs=2))
        resid = ctx.enter_context(tc.tile_pool(name="resid", bufs=1))
        psA = ctx.enter_context(tc.tile_pool(name="psA", bufs=3, space="PSUM"))
        psB = ctx.enter_context(tc.tile_pool(name="psB", bufs=2, space="PSUM"))
        psC = ctx.enter_context(tc.tile_pool(name="psC", bufs=2, space="PSUM"))

        # engine-side constants (no DMA involved)
        ident = consts.tile([128, 128], BF16, tag="ident", name="ident")
        make_identity(nc, ident)
        ones_sq = consts.tile([128, 128], BF16, tag="ones_sq", name="ones_sq")
        nc.vector.memset(ones_sq, 1.0)
        eps_t = consts.tile([128, 1], F32, tag="eps", name="eps")
        nc.vector.memset(eps_t, EPS)

        # strip 0 inputs first so phase A can start while weights stream in
        xstrip0 = xpool.tile([128, NE * 128], BF16, tag="xstrip", name="xstrip0")
        nc.sync.dma_start(out=xstrip0[:, 0:2048], in_=xT_d[0:128, 0:2048])
        nc.sync.dma_start(out=xstrip0[:, 2048:4096], in_=xT_d[0:128, 2048:4096])
        cs0 = cspool.tile([128, 512], F32, tag="cs", name="cs0")
        nc.sync.dma_start(out=cs0, in_=cs_d[0:128, :])
        bias_sb = consts.tile([128, QW + KVW], F32, tag="bias", name="bias")
        nc.sync.dma_start(out=bias_sb, in_=bias_d[:, :])

        # resident weights, posted in e-order in ~512KB chunks
        wq_sb = wpool.tile([128, NE * QW], BF16, tag="wq", name="wq")
        wkv_sb = wpool.tile([128, NE * KVW], BF16, tag="wkv", name="wkv")
        for ch in range(8):
            c0, c1 = ch * 4 * QW, (ch + 1) * 4 * QW
            nc.sync.dma_start(out=wq_sb[:, c0:c1], in_=wq_d[:, c0:c1])
            if ch % 2 == 0:
                k0, k1 = ch * 4 * KVW, (ch + 2) * 4 * KVW
                nc.sync.dma_start(out=wkv_sb[:, k0:k1], in_=wkv_d[:, k0:k1])
        mask_sb = consts.tile([128, 896], F32, tag="mask", name="mask")
        nc.sync.dma_start(out=mask_sb, in_=mask_d[:, :])

        # out-proj weights last (not needed until phase C)
        wo_sb = wpool.tile([128, HP * EMB], BF16, tag="wo", name="wo")
        nc.sync.dma_start(out=wo_sb[:, 0:HP * EMB // 2], in_=wo_d[:, 0:HP * EMB // 2])
        nc.sync.dma_start(out=wo_sb[:, HP * EMB // 2:], in_=wo_d[:, HP * EMB // 2:])

        # resident activations
        qT = [resid.tile([128, T], BF16, tag=f"qT{h}", name=f"qT{h}") for h in range(HP)]
        kT = resid.tile([128, T], BF16, tag="kT", name="kT")
        vsb = [resid.tile([128, 128], BF16, tag=f"v{j}", name=f"v{j}") for j in range(NT)]
        ctxT = [resid.tile([128, T], BF16, tag=f"ctxT{h}", name=f"ctxT{h}") for h in range(HP)]

        # ---------------- Phase A: projections + rms + rope + transpose ----
        for it in range(NT):
            if it == 0:
                xstrip, cs = xstrip0, cs0
            else:
                xstrip = xpool.tile([128, NE * 128], BF16, tag="xstrip",
                                    name=f"xstrip{it}")
                r0, r1 = it * 128, (it + 1) * 128
                nc.sync.dma_start(out=xstrip[:, 0:2048], in_=xT_d[r0:r1, 0:2048])
                nc.sync.dma_start(out=xstrip[:, 2048:4096], in_=xT_d[r0:r1, 2048:4096])
                cs = cspool.tile([128, 512], F32, tag="cs", name=f"cs{it}")
                nc.sync.dma_start(out=cs, in_=cs_d[r0:r1, :])

            q_ps = psA.tile([128, QW], F32, tag="m", name="q_ps")
            kv_ps = psB.tile([128, KVW], F32, tag="c", name="kv_ps")
            for e in range(NE):
                xt = xstrip[:, e * 128:(e + 1) * 128]
                nc.tensor.matmul(q_ps, xt, wq_sb[:, e * QW:(e + 1) * QW],
                                 start=(e == 0), stop=(e == NE - 1))
                nc.tensor.matmul(kv_ps, xt, wkv_sb[:, e * KVW:(e + 1) * KVW],
                                 start=(e == 0), stop=(e == NE - 1))
            nc.vector.tensor_add(q_ps, q_ps, bias_sb[:, 0:QW])
            nc.vector.tensor_add(kv_ps, kv_ps, bias_sb[:, QW:QW + KVW])

            for b in range(HP + 1):  # 0..3 q heads, 4 = k
                if b < HP:
                    src = q_ps[:, b * 128:(b + 1) * 128]
                    c_t, s_t = cs[:, 0:128], cs[:, 128:256]
                else:
                    src = kv_ps[:, 0:128]
                    c_t, s_t = cs[:, 256:384], cs[:, 384:512]
                sqout = scratch.tile([128, 128], F32, tag="sqout", name="sqout")
                sqacc = small.tile([128, 1], F32, tag="sqacc", name="sqacc")
                nc.scalar.activation(
                    out=sqout, in_=src,
                    func=mybir.ActivationFunctionType.Square,
                    accum_out=sqacc,
                )
                rstd = small.tile([128, 1], F32, tag="rstd", name="rstd")
                nc.scalar.activation(
                    out=rstd, in_=sqacc,
                    func=mybir.ActivationFunctionType.Sqrt,
                    bias=eps_t, scale=1.0 / D,
                )
                nc.vector.reciprocal_approx_fast(out=rstd, in_=rstd)
                # rope: out1 = x1*c1 - x2*s1 ; out2 = x2*c2 + x1*s2
                rt = scratch.tile([128, 128], F32, tag="rt", name="rt")
                m1 = scratch.tile([128, 64], F32, tag="m1", name="m1")
                nc.vector.tensor_mul(rt[:, 0:64], src[:, 0:64], c_t[:, 0:64])
                nc.vector.tensor_mul(m1, src[:, 64:128], s_t[:, 0:64])
                nc.vector.tensor_sub(rt[:, 0:64], rt[:, 0:64], m1)
                m2 = scratch.tile([128, 64], F32, tag="m2", name="m2")
                nc.vector.tensor_mul(rt[:, 64:128], src[:, 64:128], c_t[:, 64:128])
                nc.vector.tensor_mul(m2, src[:, 0:64], s_t[:, 64:128])
                nc.vector.tensor_add(rt[:, 64:128], rt[:, 64:128], m2)
                rb = scratch.tile([128, 128], BF16, tag="rb", name="rb")
                nc.vector.tensor_scalar_mul(rb, rt, rstd)
                tp = psC.tile([128, 128], BF16, tag="d", name="tp")
                nc.tensor.transpose(tp, rb, ident)
                dst = qT[b] if b < HP else kT
                nc.scalar.copy(out=dst[:, it * 128:(it + 1) * 128], in_=tp)
            # v
            nc.scalar.copy(out=vsb[it], in_=kv_ps[:, 128:256])

        # ---------------- Phase B + C interleaved per 512-query slice ------
        for si in range(T // 512):
            njb = 4 * si + 4
            blocks = [(h, jb) for h in range(HP) for jb in range(njb)]
            s_tiles = {}

            def emit_s(idx):
                h, jb = blocks[idx]
                s_ps = psA.tile([128, 512], F32, tag="m", name="s_ps")
                nc.tensor.matmul(
                    s_ps, kT[:, jb * 128:(jb + 1) * 128],
                    qT[h][:, si * 512:(si + 1) * 512],
                    start=True, stop=True,
                )
                kk = jb - 4 * si
                if kk >= 0:  # diagonal (partially masked) block
                    off = 384 - 128 * kk
                    nc.vector.tensor_add(s_ps, s_ps, mask_sb[:, off:off + 512])
                s_tiles[idx] = s_ps

            emit_s(0)
            emit_s(1)
            ctx_ps = den_ps = None
            for i, (h, jb) in enumerate(blocks):
                p_t = ppool.tile([128, 512], BF16, tag="pt", name="pt")
                nc.scalar.activation(
                    out=p_t, in_=s_tiles.pop(i),
                    func=mybir.ActivationFunctionType.Exp,
                    scale=SM_SCALE,
                )
                if i + 2 < len(blocks):
                    emit_s(i + 2)
                if jb == 0:
                    ctx_ps = psB.tile([128, 512], F32, tag="c", name="ctx_ps")
                    den_ps = psC.tile([128, 512], F32, tag="d", name="den_ps")
                nc.tensor.matmul(den_ps, ones_sq, p_t,
                                 start=(jb == 0), stop=(jb == njb - 1))
                nc.tensor.matmul(ctx_ps, vsb[jb], p_t,
                                 start=(jb == 0), stop=(jb == njb - 1))
                if jb == njb - 1:
                    rden = epool.tile([128, 512], F32, tag="rden", name="rden")
                    nc.vector.reciprocal_approx_fast(out=rden, in_=den_ps)
                    nc.vector.tensor_mul(
                        ctxT[h][:, si * 512:(si + 1) * 512], ctx_ps, rden)

            # out projection for this slice's 4 row blocks
            for it in range(4 * si, 4 * si + 4):
                osb = opool.tile([128, EMB], BF16, tag="osb", name="osb")
                for ot in range(NO):
                    o_ps = psA.tile([128, 512], F32, tag="m", name="o_ps")
                    for hh in range(HP):
                        nc.tensor.matmul(
                            o_ps,
                            ctxT[hh][:, it * 128:(it + 1) * 128],
                            wo_sb[:, hh * EMB + ot * 512:hh * EMB + (ot + 1) * 512],
                            start=(hh == 0), stop=(hh == HP - 1),
                        )
                    if ot % 2 == 0:
                        nc.scalar.copy(out=osb[:, ot * 512:(ot + 1) * 512], in_=o_ps)
                    else:
                        nc.vector.tensor_copy(
                            out=osb[:, ot * 512:(ot + 1) * 512], in_=o_ps)
                nc.sync.dma_start(
                    out=out_d[it * 128:(it + 1) * 128, :], in_=osb)

    return nc


def _prep_inputs(x, mask, cos, sin, wq, bq, wk, bk, wv, bv, wo, q_scale, k_scale):
    x2 = np.asarray(x, dtype=np.float32).reshape(T, EMB)
    # strip layout: row (it*128 + p), col (eb*128 + t) holds x[it*128+t, eb*128+p]
    xTt = x2.reshape(NT, 128, NE, 128).transpose(0, 3, 2, 1)
    xTt = np.ascontiguousarray(xTt).reshape(NT * 128, NE * 128).astype(BF)

    qs = np.asarray(q_scale, dtype=np.float32)
    ks = np.asarray(k_scale, dtype=np.float32)
    qs_rot = np.concatenate([qs[64:], qs[:64]])
    ks_rot = np.concatenate([ks[64:], ks[:64]])
    cos = np.asarray(cos, dtype=np.float32)
    sin = np.asarray(sin, dtype=np.float32)
    cs = np.concatenate([cos * qs[None, :], sin * qs_rot[None, :],
                         cos * ks[None, :], sin * ks_rot[None, :]], axis=1)
    cs = np.ascontiguousarray(cs, dtype=np.float32)

    jj = np.arange(128)[:, None]
    cc = np.arange(896)[None, :]
    maskT = np.where(jj > cc - 384, NEG, 0.0).astype(np.float32)

    wq = np.asarray(wq, dtype=np.float32)
    wk = np.asarray(wk, dtype=np.float32)
    wv = np.asarray(wv, dtype=np.float32)
    wo = np.asarray(wo, dtype=np.float32)
    bq = np.asarray(bq, dtype=np.float32)
    bk = np.asarray(bk, dtype=np.float32)
    bv = np.asarray(bv, dtype=np.float32)

    in_maps = []
    for c in range(NCORES):
        # [p, e*QW + o] = wq[e*128 + p, c*QW + o]
        wq_c = wq[:, c * QW:(c + 1) * QW].reshape(NE, 128, QW)
        wq_c = np.ascontiguousarray(wq_c.transpose(1, 0, 2)).reshape(128, NE * QW)
        wkv_c = np.concatenate(
            [wk[:, c * D:(c + 1) * D], wv[:, c * D:(c + 1) * D]], axis=1)
        wkv_c = wkv_c.reshape(NE, 128, KVW)
        wkv_c = np.ascontiguousarray(wkv_c.transpose(1, 0, 2)).reshape(128, NE * KVW)
        # [p, h*EMB + col] = wo[c*QW + h*128 + p, col]
        wo_c = wo[c * QW:(c + 1) * QW, :].reshape(HP, 128, EMB)
        wo_c = np.ascontiguousarray(wo_c.transpose(1, 0, 2)).reshape(128, HP * EMB)
        bias_c = np.broadcast_to(
            np.concatenate([bq[c * QW:(c + 1) * QW],
                            bk[c * D:(c + 1) * D], bv[c * D:(c + 1) * D]]),
            (128, QW + KVW))
        in_maps.append({
            "xT": xTt,
            "wq": wq_c.astype(BF),
            "wkv": wkv_c.astype(BF),
            "wo": wo_c.astype(BF),
            "cs": cs,
            "maskT": maskT,
            "biasb": np.ascontiguousarray(bias_c, dtype=np.float32),
        })
    return in_maps


def _get_program():
    if "nc" not in _prog_cache:
        nc = _build_program()
        if not nc.is_finalized():
            nc.finalize()
        _prog_cache["nc"] = nc
    return _prog_cache["nc"]


def kernel(**inputs):
    in_maps = _prep_inputs(**inputs)
    nc = _get_program()
    res = run_bass_kernel_spmd(nc, in_maps, list(range(NCORES)))
    out = np.zeros((T, EMB), dtype=np.float32)
    for r in res.results:
        out += np.asarray(r["out"], dtype=np.float32)
    return out.reshape(1, T, EMB)


# revision 6
# speedup vs baseline: 1.0818x; 1.0710x over previous
"""GQA attention kernel for Trainium2, tensor-parallel over heads on 8 cores.

Problem: B=1, T=2048, EMB=4096, H=32 query heads, G=8 KV groups, D=128.
Reference: q/k/v projections -> per-head RMS norm (q,k) -> RoPE (q,k) ->
causal GQA attention -> out projection.

Sharding: core c owns query heads [4c, 4c+4) and KV group c.  Each core
computes a partial output for its heads; host sums the 8 partials (the
all-reduce of the module's TP scheme, done on host since full I/O is
required anyway).

v4 (482us v3 -> target ~430us): psA bufs=4 + lookahead-3 S pipeline;
diagonal blocks narrowed to their unmasked query columns in
S/mask/exp/den/ctx; phase C copies rebalanced 5:3 ACT:DVE; opool bufs=3.

v3 (487us v2 -> target ~420us):
  - Phase B flattened: one software pipeline per 512-query slice across
    all 4 heads (S matmuls 2 blocks ahead), si-outer loop; the group
    epilogue (reciprocal+normalize) lands on the DVE queue after the
    next group's mask add, off the critical path.
  - Phase C interleaved per si-group right after its 4 head-groups:
    output DMA drains during attention instead of all at the end.
  - reciprocal_approx_fast (5x faster DVE reciprocal, 18-bit) for both
    softmax denominators and rms rstd.
  - Phase C PSUM->SBUF copies alternate between ACT and DVE.
  - Weight DMAs split into ~512KB posts interleaved in e-order so the
    first projection matmuls start as soon as their slice arrives.
"""

import numpy as np
import ml_dtypes
from contextlib import ExitStack

import concourse.bass as bass
import concourse.bacc as bacc
import concourse.mybir as mybir
from concourse.tile import TileContext
from concourse.bass_utils import run_bass_kernel_spmd
from concourse.masks import make_identity

EMB, H, G, D, T = 4096, 32, 8, 128, 2048
EPS = 1e-6
NCORES = 8
HP = H // NCORES          # 4 query heads per core
NT = T // 128             # 16 t-tiles
NE = EMB // 128           # 32 e-tiles
NO = EMB // 512           # 8 output column tiles
QW = HP * D               # 512 = q width per core
KVW = 2 * D               # 256 = k|v width per core
SM_SCALE = 1.0 / float(np.sqrt(D))
NEG = -1e9

F32 = mybir.dt.float32
BF16 = mybir.dt.bfloat16
BF = ml_dtypes.bfloat16

_prog_cache = {}


def _build_program():
    nc = bacc.Bacc()

    xT_d = nc.declare_dram_parameter("xT", [NT * 128, NE * 128], BF16, isOutput=False)
    wq_d = nc.declare_dram_parameter("wq", [128, NE * QW], BF16, isOutput=False)
    wkv_d = nc.declare_dram_parameter("wkv", [128, NE * KVW], BF16, isOutput=False)
    wo_d = nc.declare_dram_parameter("wo", [128, HP * EMB], BF16, isOutput=False)
    cs_d = nc.declare_dram_parameter("cs", [NT * 128, 512], F32, isOutput=False)
    mask_d = nc.declare_dram_parameter("maskT", [128, 896], F32, isOutput=False)
    bias_d = nc.declare_dram_parameter("biasb", [128, QW + KVW], F32, isOutput=False)
    out_d = nc.declare_dram_parameter("out", [T, EMB], BF16, isOutput=True)

    with TileContext(nc) as tc, ExitStack() as ctx:
        consts = ctx.enter_context(tc.tile_pool(name="consts", bufs=1))
        wpool = ctx.enter_context(tc.tile_pool(name="wpool", bufs=1))
        xpool = ctx.enter_context(tc.tile_pool(name="xpool", bufs=2))
        cspool = ctx.enter_context(tc.tile_pool(name="cspool", bufs=2))
        scratch = ctx.enter_context(tc.tile_pool(name="scratch", bufs=3))
        small = ctx.enter_context(tc.tile_pool(name="small", bufs=4))
        ppool = ctx.enter_context(tc.tile_pool(name="ppool", bufs=3))
        epool = ctx.enter_context(tc.tile_pool(name="epool", bufs=2))
        opool = ctx.enter_context(tc.tile_pool(name="opool", bufs=3))
        resid = ctx.enter_context(tc.tile_pool(name="resid", bufs=1))
        psA = ctx.enter_context(tc.tile_pool(name="psA", bufs=4, space="PSUM"))
        psB = ctx.enter_context(tc.tile_pool(name="psB", bufs=2, space="PSUM"))
        psC = ctx.enter_context(tc.tile_pool(name="psC", bufs=2, space="PSUM"))

        # engine-side constants (no DMA involved)
        ident = consts.tile([128, 128], BF16, tag="ident", name="ident")
        make_identity(nc, ident)
        ones_sq = consts.tile([128, 128], BF16, tag="ones_sq", name="ones_sq")
        nc.vector.memset(ones_sq, 1.0)
        eps_t = consts.tile([128, 1], F32, tag="eps", name="eps")
        nc.vector.memset(eps_t, EPS)

        # strip 0 inputs first so phase A can start while weights stream in
        xstrip0 = xpool.tile([128, NE * 128], BF16, tag="xstrip", name="xstrip0")
        nc.sync.dma_start(out=xstrip0[:, 0:2048], in_=xT_d[0:128, 0:2048])
        nc.sync.dma_start(out=xstrip0[:, 2048:4096], in_=xT_d[0:128, 2048:4096])
        cs0 = cspool.tile([128, 512], F32, tag="cs", name="cs0")
        nc.sync.dma_start(out=cs0, in_=cs_d[0:128, :])
        bias_sb = consts.tile([128, QW + KVW], F32, tag="bias", name="bias")
        nc.sync.dma_start(out=bias_sb, in_=bias_d[:, :])

        # resident weights, posted in e-order in ~512KB chunks
        wq_sb = wpool.tile([128, NE * QW], BF16, tag="wq", name="wq")
        wkv_sb = wpool.tile([128, NE * KVW], BF16, tag="wkv", name="wkv")
        for ch in range(8):
            c0, c1 = ch * 4 * QW, (ch + 1) * 4 * QW
            nc.sync.dma_start(out=wq_sb[:, c0:c1], in_=wq_d[:, c0:c1])
            if ch % 2 == 0:
                k0, k1 = ch * 4 * KVW, (ch + 2) * 4 * KVW
                nc.sync.dma_start(out=wkv_sb[:, k0:k1], in_=wkv_d[:, k0:k1])
        mask_sb = consts.tile([128, 896], F32, tag="mask", name="mask")
        nc.sync.dma_start(out=mask_sb, in_=mask_d[:, :])

        # out-proj weights last (not needed until phase C)
        wo_sb = wpool.tile([128, HP * EMB], BF16, tag="wo", name="wo")
        nc.sync.dma_start(out=wo_sb[:, 0:HP * EMB // 2], in_=wo_d[:, 0:HP * EMB // 2])
        nc.sync.dma_start(out=wo_sb[:, HP * EMB // 2:], in_=wo_d[:, HP * EMB // 2:])

        # resident activations
        qT = [resid.tile([128, T], BF16, tag=f"qT{h}", name=f"qT{h}") for h in range(HP)]
        kT = resid.tile([128, T], BF16, tag="kT", name="kT")
        vsb = [resid.tile([128, 128], BF16, tag=f"v{j}", name=f"v{j}") for j in range(NT)]
        ctxT = [resid.tile([128, T], BF16, tag=f"ctxT{h}", name=f"ctxT{h}") for h in range(HP)]

        # ---------------- Phase A: projections + rms + rope + transpose ----
        for it in range(NT):
            if it == 0:
                xstrip, cs = xstrip0, cs0
            else:
                xstrip = xpool.tile([128, NE * 128], BF16, tag="xstrip",
                                    name=f"xstrip{it}")
                r0, r1 = it * 128, (it + 1) * 128
                nc.sync.dma_start(out=xstrip[:, 0:2048], in_=xT_d[r0:r1, 0:2048])
                nc.sync.dma_start(out=xstrip[:, 2048:4096], in_=xT_d[r0:r1, 2048:4096])
                cs = cspool.tile([128, 512], F32, tag="cs", name=f"cs{it}")
                nc.sync.dma_start(out=cs, in_=cs_d[r0:r1, :])

            q_ps = psA.tile([128, QW], F32, tag="m", name="q_ps")
            kv_ps = psB.tile([128, KVW], F32, tag="c", name="kv_ps")
            for e in range(NE):
                xt = xstrip[:, e * 128:(e + 1) * 128]
                nc.tensor.matmul(q_ps, xt, wq_sb[:, e * QW:(e + 1) * QW],
                                 start=(e == 0), stop=(e == NE - 1))
                nc.tensor.matmul(kv_ps, xt, wkv_sb[:, e * KVW:(e + 1) * KVW],
                                 start=(e == 0), stop=(e == NE - 1))
            nc.vector.tensor_add(q_ps, q_ps, bias_sb[:, 0:QW])
            nc.vector.tensor_add(kv_ps, kv_ps, bias_sb[:, QW:QW + KVW])

            for b in range(HP + 1):  # 0..3 q heads, 4 = k
                if b < HP:
                    src = q_ps[:, b * 128:(b + 1) * 128]
                    c_t, s_t = cs[:, 0:128], cs[:, 128:256]
                else:
                    src = kv_ps[:, 0:128]
                    c_t, s_t = cs[:, 256:384], cs[:, 384:512]
                sqout = scratch.tile([128, 128], F32, tag="sqout", name="sqout")
                sqacc = small.tile([128, 1], F32, tag="sqacc", name="sqacc")
                nc.scalar.activation(
                    out=sqout, in_=src,
                    func=mybir.ActivationFunctionType.Square,
                    accum_out=sqacc,
                )
                rstd = small.tile([128, 1], F32, tag="rstd", name="rstd")
                nc.scalar.activation(
                    out=rstd, in_=sqacc,
                    func=mybir.ActivationFunctionType.Sqrt,
                    bias=eps_t, scale=1.0 / D,
                )
                nc.vector.reciprocal_approx_fast(out=rstd, in_=rstd)
                # rope: out1 = x1*c1 - x2*s1 ; out2 = x2*c2 + x1*s2
                rt = scratch.tile([128, 128], F32, tag="rt", name="rt")
                m1 = scratch.tile([128, 64], F32, tag="m1", name="m1")
                nc.vector.tensor_mul(rt[:, 0:64], src[:, 0:64], c_t[:, 0:64])
                nc.vector.tensor_mul(m1, src[:, 64:128], s_t[:, 0:64])
                nc.vector.tensor_sub(rt[:, 0:64], rt[:, 0:64], m1)
                m2 = scratch.tile([128, 64], F32, tag="m2", name="m2")
                nc.vector.tensor_mul(rt[:, 64:128], src[:, 64:128], c_t[:, 64:128])
                nc.vector.tensor_mul(m2, src[:, 0:64], s_t[:, 64:128])
                nc.vector.tensor_add(rt[:, 64:128], rt[:, 64:128], m2)
                rb = scratch.tile([128, 128], BF16, tag="rb", name="rb")
                nc.vector.tensor_scalar_mul(rb, rt, rstd)
                tp = psC.tile([128, 128], BF16, tag="d", name="tp")
                nc.tensor.transpose(tp, rb, ident)
                dst = qT[b] if b < HP else kT
                nc.scalar.copy(out=dst[:, it * 128:(it + 1) * 128], in_=tp)
            # v
            nc.scalar.copy(out=vsb[it], in_=kv_ps[:, 128:256])

        # ---------------- Phase B + C interleaved per 512-query slice ------
        LOOKAHEAD = 3
        for si in range(T // 512):
            njb = 4 * si + 4
            blocks = [(h, jb) for h in range(HP) for jb in range(njb)]
            s_tiles = {}

            def emit_s(idx):
                h, jb = blocks[idx]
                kk = jb - 4 * si
                # masked-out query columns of diagonal blocks are skipped
                q0 = 128 * kk if kk > 0 else 0
                s_ps = psA.tile([128, 512], F32, tag="m", name="s_ps")
                nc.tensor.matmul(
                    s_ps[:, q0:512], kT[:, jb * 128:(jb + 1) * 128],
                    qT[h][:, si * 512 + q0:(si + 1) * 512],
                    start=True, stop=True,
                )
                if kk >= 0:  # diagonal (partially masked) block
                    nc.vector.tensor_add(
                        s_ps[:, q0:512], s_ps[:, q0:512],
                        mask_sb[:, 384:384 + 512 - q0])
                s_tiles[idx] = (s_ps, q0)

            for idx in range(min(LOOKAHEAD, len(blocks))):
                emit_s(idx)
            ctx_ps = den_ps = None
            for i, (h, jb) in enumerate(blocks):
                s_ps, q0 = s_tiles.pop(i)
                p_t = ppool.tile([128, 512], BF16, tag="pt", name="pt")
                nc.scalar.activation(
                    out=p_t[:, q0:512], in_=s_ps[:, q0:512],
                    func=mybir.ActivationFunctionType.Exp,
                    scale=SM_SCALE,
                )
                if i + LOOKAHEAD < len(blocks):
                    emit_s(i + LOOKAHEAD)
                if jb == 0:
                    ctx_ps = psB.tile([128, 512], F32, tag="c", name="ctx_ps")
                    den_ps = psC.tile([128, 512], F32, tag="d", name="den_ps")
                nc.tensor.matmul(den_ps[:, q0:512], ones_sq, p_t[:, q0:512],
                                 start=(jb == 0), stop=(jb == njb - 1),
                                 skip_group_check=True)
                nc.tensor.matmul(ctx_ps[:, q0:512], vsb[jb], p_t[:, q0:512],
                                 start=(jb == 0), stop=(jb == njb - 1),
                                 skip_group_check=True)
                if jb == njb - 1:
                    rden = epool.tile([128, 512], F32, tag="rden", name="rden")
                    nc.vector.reciprocal_approx_fast(out=rden, in_=den_ps)
                    nc.vector.tensor_mul(
                        ctxT[h][:, si * 512:(si + 1) * 512], ctx_ps, rden)

            # out projection for this slice's 4 row blocks
            for it in range(4 * si, 4 * si + 4):
                osb = opool.tile([128, EMB], BF16, tag="osb", name="osb")
                for ot in range(NO):
                    o_ps = psA.tile([128, 512], F32, tag="m", name="o_ps")
                    for hh in range(HP):
                        nc.tensor.matmul(
                            o_ps,
                            ctxT[hh][:, it * 128:(it + 1) * 128],
                            wo_sb[:, hh * EMB + ot * 512:hh * EMB + (ot + 1) * 512],
                            start=(hh == 0), stop=(hh == HP - 1),
                        )
                    if ot % 3 == 0:
                        nc.vector.tensor_copy(
                            out=osb[:, ot * 512:(ot + 1) * 512], in_=o_ps)
                    else:
                        nc.scalar.copy(out=osb[:, ot * 512:(ot + 1) * 512], in_=o_ps)
                nc.sync.dma_start(
                    out=out_d[it * 128:(it + 1) * 128, :], in_=osb)

    return nc


def _prep_inputs(x, mask, cos, sin, wq, bq, wk, bk, wv, bv, wo, q_scale, k_scale):
    x2 = np.asarray(x, dtype=np.float32).reshape(T, EMB)
    # strip layout: row (it*128 + p), col (eb*128 + t) holds x[it*128+t, eb*128+p]
    xTt = x2.reshape(NT, 128, NE, 128).transpose(0, 3, 2, 1)
    xTt = np.ascontiguousarray(xTt).reshape(NT * 128, NE * 128).astype(BF)

    qs = np.asarray(q_scale, dtype=np.float32)
    ks = np.asarray(k_scale, dtype=np.float32)
    qs_rot = np.concatenate([qs[64:], qs[:64]])
    ks_rot = np.concatenate([ks[64:], ks[:64]])
    cos = np.asarray(cos, dtype=np.float32)
    sin = np.asarray(sin, dtype=np.float32)
    cs = np.concatenate([cos * qs[None, :], sin * qs_rot[None, :],
                         cos * ks[None, :], sin * ks_rot[None, :]], axis=1)
    cs = np.ascontiguousarray(cs, dtype=np.float32)

    jj = np.arange(128)[:, None]
    cc = np.arange(896)[None, :]
    maskT = np.where(jj > cc - 384, NEG, 0.0).astype(np.float32)

    wq = np.asarray(wq, dtype=np.float32)
    wk = np.asarray(wk, dtype=np.float32)
    wv = np.asarray(wv, dtype=np.float32)
    wo = np.asarray(wo, dtype=np.float32)
    bq = np.asarray(bq, dtype=np.float32)
    bk = np.asarray(bk, dtype=np.float32)
    bv = np.asarray(bv, dtype=np.float32)

    in_maps = []
    for c in range(NCORES):
        # [p, e*QW + o] = wq[e*128 + p, c*QW + o]
        wq_c = wq[:, c * QW:(c + 1) * QW].reshape(NE, 128, QW)
        wq_c = np.ascontiguousarray(wq_c.transpose(1, 0, 2)).reshape(128, NE * QW)
        wkv_c = np.concatenate(
            [wk[:, c * D:(c + 1) * D], wv[:, c * D:(c + 1) * D]], axis=1)
        wkv_c = wkv_c.reshape(NE, 128, KVW)
        wkv_c = np.ascontiguousarray(wkv_c.transpose(1, 0, 2)).reshape(128, NE * KVW)
        # [p, h*EMB + col] = wo[c*QW + h*128 + p, col]
        wo_c = wo[c * QW:(c + 1) * QW, :].reshape(HP, 128, EMB)
        wo_c = np.ascontiguousarray(wo_c.transpose(1, 0, 2)).reshape(128, HP * EMB)
        bias_c = np.broadcast_to(
            np.concatenate([bq[c * QW:(c + 1) * QW],
                            bk[c * D:(c + 1) * D], bv[c * D:(c + 1) * D]]),
            (128, QW + KVW))
        in_maps.append({
            "xT": xTt,
            "wq": wq_c.astype(BF),
            "wkv": wkv_c.astype(BF),
            "wo": wo_c.astype(BF),
            "cs": cs,
            "maskT": maskT,
            "biasb": np.ascontiguousarray(bias_c, dtype=np.float32),
        })
    return in_maps


def _get_program():
    if "nc" not in _prog_cache:
        nc = _build_program()
        if not nc.is_finalized():
            nc.finalize()
        _prog_cache["nc"] = nc
    return _prog_cache["nc"]


def kernel(**inputs):
    in_maps = _prep_inputs(**inputs)
    nc = _get_program()
    res = run_bass_kernel_spmd(nc, in_maps, list(range(NCORES)))
    out = np.zeros((T, EMB), dtype=np.float32)
    for r in res.results:
        out += np.asarray(r["out"], dtype=np.float32)
    return out.reshape(1, T, EMB)
